# revision 1
# baseline (speedup 1.0000x reference)
"""Trainium2 Bass kernel for sparse 3D voxel convolution (e3nn-style, 5^3 taps).

Sharding: data-parallel over the N=200000 sparse voxels. Voxels are sorted by
x-plane and split into 8 contiguous slabs of 25000 destination voxels; each
core gets a local feature table holding its slab plus the +/-2-plane halo
(<32k rows, so int16 gather indices work). The tiny kernel-generation weights
(8x2304) and residual Linear weights are folded on host into 125 tap matrices
K[80,80] (residual into the center tap), replicated to all cores.

Device pipeline per core:
  - center tap (every voxel, src==dst): direct HWDGE DMA load of 128-row
    blocks -> PE transpose -> matmul(lhsT=X^T, rhs=K62) -> direct store.
  - 124 sparse taps: host-planned pair stream (columns of 128 pairs, padded
    per tap, identical column->tap schedule on all cores). dma_gather
    (SWDGE custom op, int16 local ids) -> PE transpose -> matmul ->
    dma_scatter_add into the output table. Scatter instructions never cross
    tap boundaries (destinations are unique within a tap; duplicates within
    one scatter instruction lose updates on HW). Pad slots gather row 0 and
    scatter into a trash row past the real output rows.
"""

import sys
import types

import numpy as np

NRB = 8
RAD = 2.5
GRID = 192
N = 200000
DIM = 80
EP = 128                       # padded feature row (512B) for dma_gather
ALPHA = 1.0 / np.sqrt(48.0)
N_CORES = 8
N_LOC = N // N_CORES           # 25000 dst voxels per core
CJ = 16                        # columns per center group (direct DMA path)
NCEN = 13 * CJ                 # 208 center columns -> covers rows 0..26623
CEN_ROWS = NCEN * 128          # 26624
TRASH = CEN_ROWS               # scatter trash row
NT = CEN_ROWS + 128            # out table rows
JMAX = 8                       # max columns per gather/scatter instruction
                               # (custom-op ucode fails above 1024 indices)
SUB = 4                        # columns per PSUM bank group

_ax = np.arange(-2.0, 3.0, dtype=np.float32)
LATTICE = np.stack(np.meshgrid(_ax, _ax, _ax, indexing="ij"), -1)
PERM = np.arange(125).reshape(5, 5, 5).transpose(2, 1, 0).reshape(-1)
OFFS = LATTICE.reshape(-1, 3).astype(np.int32)[PERM]
CENTER_TAP = 62


def _radial_emb():
    d = np.linalg.norm(LATTICE, axis=-1)
    centers = np.linspace(0.0, RAD, NRB + 2)[1:-1]
    step = centers[1] - centers[0]
    t = (d[..., None] - centers) / step
    inside = np.abs(t) < 1.0
    safe = np.where(inside, 1.0 - t * t, 1.0)
    return (1.14136 * np.exp(2.0) * np.where(inside, np.exp(-2.0 / safe), 0.0)).astype(
        np.float32
    )


EMB = _radial_emb().reshape(-1, NRB)


def _sph():
    n = np.linalg.norm(LATTICE, axis=-1, keepdims=True)
    u = np.where(n > 0, LATTICE / np.maximum(n, 1e-9), 0.0)
    return np.concatenate([np.ones_like(n), np.sqrt(3.0) * u], -1).astype(np.float32)


SH = _sph().reshape(-1, 4)


def make_kernel_np(weight):
    w = (EMB @ weight.astype(np.float32)) / 125.0
    w1 = w[:, :1024].reshape(125, 32, 32)
    w2 = w[:, 1024:1536].reshape(125, 32, 16)
    w3 = w[:, 1536:1792].reshape(125, 16, 16)
    w4 = w[:, 1792:].reshape(125, 16, 32)
    s0 = SH[:, 0]
    v = SH[:, 1:]
    eye3 = np.eye(3, dtype=w.dtype)
    K00 = ALPHA * w1 * s0[:, None, None]
    K01 = ALPHA * np.einsum("pik,pm->pikm", w2, v).reshape(125, 32, 48)
    K11 = ALPHA * np.einsum(
        "pik,mn->pimkn", w3 * s0[:, None, None], eye3
    ).reshape(125, 48, 48)
    K10 = (ALPHA / np.sqrt(3.0)) * np.einsum("pik,pm->pimk", w4, v).reshape(125, 48, 32)
    K = np.concatenate(
        [np.concatenate([K00, K01], 2), np.concatenate([K10, K11], 2)], 1
    )
    return K[PERM]


def w_sc_embed(w_sc0, w_sc1):
    W = np.zeros((80, 80), np.float32)
    W[:32, :32] = w_sc0 / np.sqrt(32.0)
    blk = np.zeros((48, 48), np.float32)
    for m in range(3):
        blk[m::3, m::3] = w_sc1 / np.sqrt(16.0)
    W[32:, 32:] = blk
    return W


def build_pairs(coords):
    idx_vol = np.full(GRID * GRID * GRID, -1, np.int32)
    lin = (coords[:, 0].astype(np.int64) * GRID + coords[:, 1]) * GRID + coords[:, 2]
    idx_vol[lin] = np.arange(N, dtype=np.int32)
    all_i = np.arange(N, dtype=np.int32)
    dsts, srcs = [], []
    for t in range(125):
        if t == CENTER_TAP:
            dsts.append(None)
            srcs.append(None)
            continue
        c = coords + OFFS[t]
        ok = np.all((c >= 0) & (c < GRID), axis=1)
        cl = (c[:, 0].astype(np.int64) * GRID + c[:, 1]) * GRID + c[:, 2]
        cl = np.clip(cl, 0, GRID**3 - 1)
        nb = idx_vol[cl]
        valid = ok & (nb >= 0)
        dsts.append(all_i[valid])
        srcs.append(nb[valid])
    return dsts, srcs


def wrap16(a):
    """Token stream [n] -> [128, n//16] int16 (16-partition wrap, 8x replicated)."""
    n = a.shape[0]
    w = a.reshape(n // 16, 16).T
    return np.ascontiguousarray(np.tile(w, (8, 1)).astype(np.int16))


def build_plan(feats, coords):
    """Returns (feats_loc [8,SRC_ROWS,EP], gidx_w [8,128,8W], sidx_w [8,128,8W],
    groups, order) where groups is a list of
    (ncols, [(tap, col_lo, col_hi), ...]) shared by all cores."""
    order = np.argsort(coords[:, 0], kind="stable").astype(np.int32)
    pos = np.empty(N, np.int32)
    pos[order] = np.arange(N, dtype=np.int32)
    core_of = pos // N_LOC
    loc_dst = pos % N_LOC

    dsts, srcs = build_pairs(coords)
    taps = [t for t in range(125) if t != CENTER_TAP]

    # per (core, tap) local pair lists
    per_core = [[None] * 125 for _ in range(N_CORES)]
    for t in taps:
        d, s = dsts[t], srcs[t]
        cd = core_of[d]
        for c in range(N_CORES):
            m = cd == c
            dl = loc_dst[d[m]]
            sg = s[m]
            o = np.argsort(dl, kind="stable")
            per_core[c][t] = (dl[o], sg[o])

    # local src tables
    SRC_MIN = CEN_ROWS
    glob2loc = np.full((N_CORES, N), -1, np.int32)
    extras = []
    for c in range(N_CORES):
        dg = order[c * N_LOC : (c + 1) * N_LOC]
        glob2loc[c, dg] = np.arange(N_LOC, dtype=np.int32)
        need = np.unique(np.concatenate([per_core[c][t][1] for t in taps]))
        ex = need[glob2loc[c, need] < 0]
        glob2loc[c, ex] = N_LOC + np.arange(len(ex), dtype=np.int32)
        extras.append(ex)
    n_src = [N_LOC + len(e) for e in extras]
    SRC_ROWS = max(SRC_MIN, max(n_src))
    assert SRC_ROWS <= 32767, n_src
    feats_loc = np.zeros((N_CORES, SRC_ROWS, EP), np.float32)
    for c in range(N_CORES):
        dg = order[c * N_LOC : (c + 1) * N_LOC]
        feats_loc[c, :N_LOC, :DIM] = feats[dg]
        feats_loc[c, N_LOC : n_src[c], :DIM] = feats[extras[c]]

    # columns per tap (max over cores); fixed JMAX-column windows, taps may
    # span windows (scatter slices stay within one tap -> unique dsts)
    w_t = {
        t: max(
            1,
            max((len(per_core[c][t][0]) + 127) // 128 for c in range(N_CORES)),
        )
        for t in taps
    }
    col_tap = []
    for t in taps:
        col_tap += [t] * w_t[t]
    W = sum(w_t.values())
    groups = []
    col = 0
    while col < W:
        wg = min(JMAX, W - col)
        runs = []
        a = 0
        while a < wg:
            t = col_tap[col + a]
            b = a
            while b < wg and col_tap[col + b] == t:
                b += 1
            runs.append((t, a, b))
            a = b
        groups.append((wg, runs))
        col += wg
    gidx = np.zeros((N_CORES, W * 128), np.int32)
    sidx = np.full((N_CORES, W * 128), TRASH, np.int32)
    col = 0
    for t in taps:
        for c in range(N_CORES):
            dl, sg = per_core[c][t]
            m = len(dl)
            a = col * 128
            gidx[c, a : a + m] = glob2loc[c, sg]
            sidx[c, a : a + m] = dl
        col += w_t[t]
    assert col == W

    # token i within its 128-col block: column-major packing (token = c*128+p)
    gidx_w = np.stack([wrap16(gidx[c]) for c in range(N_CORES)])
    sidx_w = np.stack([wrap16(sidx[c]) for c in range(N_CORES)])
    return feats_loc, gidx_w, sidx_w, groups, order, SRC_ROWS


def _install_axon_profile_hook():
    try:
        import antenv

        if "antenv.axon_hooks" not in sys.modules:
            mod = types.ModuleType("antenv.axon_hooks")
            hook = [None]
            mod.set_axon_ntff_profile_hook = lambda h: hook.__setitem__(0, h)
            mod.get_axon_ntff_profile_hook = lambda: hook[0]
            sys.modules["antenv.axon_hooks"] = mod
            antenv.axon_hooks = mod
        from antenv.axon_hooks import (
            get_axon_ntff_profile_hook,
            set_axon_ntff_profile_hook,
        )

        if get_axon_ntff_profile_hook() is None:
            from trn_agent_boot.trn_boot import _ntff_profile_via_ctypes

            set_axon_ntff_profile_hook(
                _ntff_profile_via_ctypes("/opt/axon/libaxon_pjrt.so")
            )
    except Exception:
        pass


def build_program(groups, SRC_ROWS, W, do_center=True, ngroups=None, do_scatter=True):
    import os
    n_queues = int(os.environ.get("K_QUEUES", "4"))
    n_tables = int(os.environ.get("K_TABLES", "2"))
    import concourse.bacc as bacc
    import concourse.mybir as mybir
    import concourse.tile as tile
    from concourse.masks import make_identity

    nc = bacc.Bacc(
        "TRN2", num_devices=N_CORES, debug=False, target_bir_lowering=False,
        num_swdge_queues=n_queues,
    )
    f32 = mybir.dt.float32
    i16 = mybir.dt.int16

    feats_d = nc.dram_tensor("feats_loc", [SRC_ROWS, EP], f32, kind="ExternalInput").ap()
    ktaps_d = nc.dram_tensor("ktaps", [80, 125 * 80], f32, kind="ExternalInput").ap()
    gidx_d = nc.dram_tensor("gidx", [128, 8 * W], i16, kind="ExternalInput").ap()
    sidx_d = nc.dram_tensor("sidx", [128, 8 * W], i16, kind="ExternalInput").ap()
    out_d = nc.dram_tensor("out", [NT, EP], f32, kind="ExternalOutput").ap()
    tbl = [
        nc.dram_tensor(f"tbl{i}", [NT, EP], f32, kind="ExternalOutput").ap()
        for i in range(n_tables)
    ]

    with tile.TileContext(nc) as tc:
        with (
            tc.tile_pool(name="const", bufs=1) as cpool,
            tc.tile_pool(name="gath", bufs=3) as gpool,
            tc.tile_pool(name="xts", bufs=4) as xpool,
            tc.tile_pool(name="ysb", bufs=3) as ypool,
            tc.tile_pool(name="xtp", bufs=4, space="PSUM") as xppool,
            tc.tile_pool(name="ypp", bufs=4, space="PSUM") as yppool,
        ):
            ident = cpool.tile([128, 128], f32)
            make_identity(nc, ident[:])
            ksb = cpool.tile([80, 125 * 80], f32)
            nc.sync.dma_start(out=ksb[:], in_=ktaps_d[:])
            gsb = cpool.tile([128, 8 * W], i16)
            nc.sync.dma_start(out=gsb[:], in_=gidx_d[:])
            ssb = cpool.tile([128, 8 * W], i16)
            nc.sync.dma_start(out=ssb[:], in_=sidx_d[:])

            def compute_block(G, wg, col_taps, Y):
                """G [128, wg, EP] gathered block -> Y [128, wg, DIM]."""
                for sb in range((wg + SUB - 1) // SUB):
                    k0 = sb * SUB
                    kn = min(SUB, wg - k0)
                    xt_ps = xppool.tile([80, kn, 128], f32, tag="xtp")
                    for k in range(kn):
                        nc.tensor.transpose(
                            out=xt_ps[:, k, :],
                            in_=G[:, k0 + k, :DIM],
                            identity=ident[:],
                        )
                    xt_sb = xpool.tile([80, kn, 128], f32, tag="xts")
                    nc.vector.tensor_copy(out=xt_sb[:], in_=xt_ps[:])
                    y_ps = yppool.tile([128, kn, DIM], f32, tag="ypp")
                    for k in range(kn):
                        t = col_taps[k0 + k]
                        nc.tensor.matmul(
                            out=y_ps[:, k, :],
                            lhsT=xt_sb[:, k, :],
                            rhs=ksb[:, t * 80 : (t + 1) * 80],
                            start=True,
                            stop=True,
                        )
                    nc.vector.tensor_copy(out=Y[:, k0 : k0 + kn, :], in_=y_ps[:])

            # ---- center tap: direct DMA both ways --------------------------
            for g in range(NCEN // CJ if do_center else 0):
                r0 = g * CJ * 128
                G = gpool.tile([128, CJ, EP], f32, tag="G")
                nc.sync.dma_start(
                    out=G[:],
                    in_=feats_d[r0 : r0 + CJ * 128, :].rearrange(
                        "(c p) f -> p c f", p=128
                    ),
                )
                Y = ypool.tile([128, CJ, DIM], f32, tag="Y")
                compute_block(G, CJ, [CENTER_TAP] * CJ, Y)
                nc.sync.dma_start(
                    out=out_d[r0 : r0 + CJ * 128, :DIM].rearrange(
                        "(c p) f -> p c f", p=128
                    ),
                    in_=Y[:],
                )

            # ---- sparse taps: dma_gather + per-tap dma_scatter_add ---------
            n_scat = 0
            col = 0
            for gi_, (wg, gtaps) in enumerate(groups):
                if ngroups is not None and gi_ >= ngroups:
                    break
                i0 = col * 8
                G = gpool.tile([128, wg, EP], f32, tag="G")
                nc.gpsimd.dma_gather(
                    out_ap=G[:],
                    in_ap=feats_d[:],
                    idxs_ap=gsb[:, i0 : i0 + 8 * wg],
                    num_idxs=wg * 128,
                    num_idxs_reg=wg * 128,
                    elem_size=EP,
                    queue_num=(2 * (gi_ % 2)) % n_queues,
                )
                col_taps = []
                for t, a, b in gtaps:
                    col_taps += [t] * (b - a)
                Y = ypool.tile([128, wg, DIM], f32, tag="Y")
                compute_block(G, wg, col_taps, Y)
                for t, a, b in (gtaps if do_scatter else []):
                    nc.gpsimd.dma_scatter_add(
                        out_ap=tbl[n_scat % n_tables][:, :DIM],
                        in_ap=Y[:, a:b, :],
                        idxs_ap=ssb[:, i0 + 8 * a : i0 + 8 * b],
                        num_idxs=(b - a) * 128,
                        num_idxs_reg=(b - a) * 128,
                        elem_size=DIM,
                        elem_step=EP,
                        queue_num=(1 + 2 * (n_scat % n_tables)) % n_queues if n_queues > 1 else 0,
                    )
                    n_scat += 1
                col += wg
    print("tile build done", file=sys.stderr)
    nc.compile()
    print("bacc compile done", file=sys.stderr)
    return nc


_LAST = {"exec_time_ns": None, "results": None}


def kernel(feats, weight, w_sc0, w_sc1, coords):
    feats = np.ascontiguousarray(np.asarray(feats, np.float32))
    weight = np.asarray(weight, np.float32)
    w_sc0 = np.asarray(w_sc0, np.float32)
    w_sc1 = np.asarray(w_sc1, np.float32)
    coords = np.asarray(coords, np.int32)

    K = make_kernel_np(weight)
    K[CENTER_TAP] = K[CENTER_TAP] + w_sc_embed(w_sc0, w_sc1)
    ktaps = np.ascontiguousarray(K.transpose(1, 0, 2).reshape(80, 125 * 80))

    feats_loc, gidx_w, sidx_w, groups, order, SRC_ROWS = build_plan(feats, coords)
    W = gidx_w.shape[2] // 8
    print(f"plan: W={W} groups={len(groups)} SRC_ROWS={SRC_ROWS}", file=sys.stderr)

    _install_axon_profile_hook()
    from concourse.bass_utils import run_bass_kernel_spmd

    nc = build_program(groups, SRC_ROWS, W)
    in_maps = [
        {
            "feats_loc": feats_loc[c],
            "ktaps": ktaps,
            "gidx": gidx_w[c],
            "sidx": sidx_w[c],
        }
        for c in range(N_CORES)
    ]
    import os

    trace = os.environ.get("BASS_KERNEL_TRACE", "0") == "1"
    import time as _time

    res = None
    last_exc = None
    for attempt in range(4):
        try:
            res = run_bass_kernel_spmd(
                nc,
                in_maps,
                core_ids=list(range(N_CORES)),
                trace=trace and attempt == 0,
            )
            break
        except Exception as e:  # device flake: retry, last attempts untraced
            last_exc = e
            print(f"run attempt {attempt} failed: {e}", file=sys.stderr)
            _time.sleep(3.0)
    if res is None:
        raise last_exc
    print("hw run done", file=sys.stderr)
    _LAST["exec_time_ns"] = res.exec_time_ns
    _LAST["results"] = res
    out = np.empty((N, DIM), np.float32)
    for c in range(N_CORES):
        out[order[c * N_LOC : (c + 1) * N_LOC]] = np.asarray(
            res.results[c]["out"]
        )[:N_LOC, :DIM]
    return out



# revision 5
# speedup vs baseline: 5.7141x; 5.7141x over previous
"""Trainium2 Bass kernel for sparse 3D voxel convolution (e3nn-style, 5^3 taps).

Sharding: data-parallel over the N=200000 sparse voxels. Voxels are sorted by
x-plane and split into 8 contiguous slabs of 25000 destination voxels; each
core gets a local feature table holding its slab plus the +/-2-plane halo
(<32k rows, so int16 gather indices work). The tiny kernel-generation weights
(8x2304) and residual Linear weights are folded on host into 125 tap matrices
K[80,80] (residual into the center tap), replicated to all cores.

Device pipeline per core:
  - center tap (every voxel, src==dst): direct HWDGE DMA load of 128-row
    blocks -> PE transpose -> matmul(lhsT=X^T, rhs=K62) -> direct store.
  - 124 sparse taps: host-planned pair stream (columns of 128 pairs, padded
    per tap, identical column->tap schedule on all cores). dma_gather
    (SWDGE custom op, int16 local ids) -> PE transpose -> matmul ->
    dma_scatter_add into the output table. Scatter instructions never cross
    tap boundaries (destinations are unique within a tap; duplicates within
    one scatter instruction lose updates on HW). Pad slots gather row 0 and
    scatter into a trash row past the real output rows.
"""

import sys
import types

import numpy as np

NRB = 8
RAD = 2.5
GRID = 192
N = 200000
DIM = 80
EP = 128                       # padded feature row (512B) for dma_gather
ALPHA = 1.0 / np.sqrt(48.0)
N_CORES = 8
N_LOC = N // N_CORES           # 25000 dst voxels per core
CJ = 16                        # columns per center group (direct DMA path)
NCEN = 13 * CJ                 # 208 center columns -> covers rows 0..26623
CEN_ROWS = NCEN * 128          # 26624
TRASH = CEN_ROWS               # scatter trash row
NT = CEN_ROWS + 128            # out table rows
JMAX = 8                       # max columns per gather/scatter instruction
                               # (custom-op ucode fails above 1024 indices)
SUB = 4                        # columns per PSUM bank group

_ax = np.arange(-2.0, 3.0, dtype=np.float32)
LATTICE = np.stack(np.meshgrid(_ax, _ax, _ax, indexing="ij"), -1)
PERM = np.arange(125).reshape(5, 5, 5).transpose(2, 1, 0).reshape(-1)
OFFS = LATTICE.reshape(-1, 3).astype(np.int32)[PERM]
CENTER_TAP = 62


def _radial_emb():
    d = np.linalg.norm(LATTICE, axis=-1)
    centers = np.linspace(0.0, RAD, NRB + 2)[1:-1]
    step = centers[1] - centers[0]
    t = (d[..., None] - centers) / step
    inside = np.abs(t) < 1.0
    safe = np.where(inside, 1.0 - t * t, 1.0)
    return (1.14136 * np.exp(2.0) * np.where(inside, np.exp(-2.0 / safe), 0.0)).astype(
        np.float32
    )


EMB = _radial_emb().reshape(-1, NRB)


def _sph():
    n = np.linalg.norm(LATTICE, axis=-1, keepdims=True)
    u = np.where(n > 0, LATTICE / np.maximum(n, 1e-9), 0.0)
    return np.concatenate([np.ones_like(n), np.sqrt(3.0) * u], -1).astype(np.float32)


SH = _sph().reshape(-1, 4)


def make_kernel_np(weight):
    w = (EMB @ weight.astype(np.float32)) / 125.0
    w1 = w[:, :1024].reshape(125, 32, 32)
    w2 = w[:, 1024:1536].reshape(125, 32, 16)
    w3 = w[:, 1536:1792].reshape(125, 16, 16)
    w4 = w[:, 1792:].reshape(125, 16, 32)
    s0 = SH[:, 0]
    v = SH[:, 1:]
    eye3 = np.eye(3, dtype=w.dtype)
    K00 = ALPHA * w1 * s0[:, None, None]
    K01 = ALPHA * np.einsum("pik,pm->pikm", w2, v).reshape(125, 32, 48)
    K11 = ALPHA * np.einsum(
        "pik,mn->pimkn", w3 * s0[:, None, None], eye3
    ).reshape(125, 48, 48)
    K10 = (ALPHA / np.sqrt(3.0)) * np.einsum("pik,pm->pimk", w4, v).reshape(125, 48, 32)
    K = np.concatenate(
        [np.concatenate([K00, K01], 2), np.concatenate([K10, K11], 2)], 1
    )
    return K[PERM]


def w_sc_embed(w_sc0, w_sc1):
    W = np.zeros((80, 80), np.float32)
    W[:32, :32] = w_sc0 / np.sqrt(32.0)
    blk = np.zeros((48, 48), np.float32)
    for m in range(3):
        blk[m::3, m::3] = w_sc1 / np.sqrt(16.0)
    W[32:, 32:] = blk
    return W


def build_pairs(coords):
    idx_vol = np.full(GRID * GRID * GRID, -1, np.int32)
    lin = (coords[:, 0].astype(np.int64) * GRID + coords[:, 1]) * GRID + coords[:, 2]
    idx_vol[lin] = np.arange(N, dtype=np.int32)
    all_i = np.arange(N, dtype=np.int32)
    dsts, srcs = [], []
    for t in range(125):
        if t == CENTER_TAP:
            dsts.append(None)
            srcs.append(None)
            continue
        c = coords + OFFS[t]
        ok = np.all((c >= 0) & (c < GRID), axis=1)
        cl = (c[:, 0].astype(np.int64) * GRID + c[:, 1]) * GRID + c[:, 2]
        cl = np.clip(cl, 0, GRID**3 - 1)
        nb = idx_vol[cl]
        valid = ok & (nb >= 0)
        dsts.append(all_i[valid])
        srcs.append(nb[valid])
    return dsts, srcs


def wrap16(a):
    """Token stream [n] -> [128, n//16] int16 (16-partition wrap, 8x replicated)."""
    n = a.shape[0]
    w = a.reshape(n // 16, 16).T
    return np.ascontiguousarray(np.tile(w, (8, 1)).astype(np.int16))


def build_plan(feats, coords):
    """Returns (feats_loc [8,SRC_ROWS,EP], gidx_w [8,128,8W], sidx_w [8,128,8W],
    groups, order) where groups is a list of
    (ncols, [(tap, col_lo, col_hi), ...]) shared by all cores."""
    order = np.argsort(coords[:, 0], kind="stable").astype(np.int32)
    pos = np.empty(N, np.int32)
    pos[order] = np.arange(N, dtype=np.int32)
    core_of = pos // N_LOC
    loc_dst = pos % N_LOC

    dsts, srcs = build_pairs(coords)
    taps = [t for t in range(125) if t != CENTER_TAP]

    # per (core, tap) local pair lists
    per_core = [[None] * 125 for _ in range(N_CORES)]
    for t in taps:
        d, s = dsts[t], srcs[t]
        cd = core_of[d]
        for c in range(N_CORES):
            m = cd == c
            dl = loc_dst[d[m]]
            sg = s[m]
            o = np.argsort(dl, kind="stable")
            per_core[c][t] = (dl[o], sg[o])

    # local src tables
    SRC_MIN = CEN_ROWS
    glob2loc = np.full((N_CORES, N), -1, np.int32)
    extras = []
    for c in range(N_CORES):
        dg = order[c * N_LOC : (c + 1) * N_LOC]
        glob2loc[c, dg] = np.arange(N_LOC, dtype=np.int32)
        need = np.unique(np.concatenate([per_core[c][t][1] for t in taps]))
        ex = need[glob2loc[c, need] < 0]
        glob2loc[c, ex] = N_LOC + np.arange(len(ex), dtype=np.int32)
        extras.append(ex)
    n_src = [N_LOC + len(e) for e in extras]
    SRC_ROWS = max(SRC_MIN, max(n_src))
    assert SRC_ROWS <= 32767, n_src
    feats_loc = np.zeros((N_CORES, SRC_ROWS, EP), np.float32)
    for c in range(N_CORES):
        dg = order[c * N_LOC : (c + 1) * N_LOC]
        feats_loc[c, :N_LOC, :DIM] = feats[dg]
        feats_loc[c, N_LOC : n_src[c], :DIM] = feats[extras[c]]

    # columns per tap (max over cores); fixed JMAX-column windows, taps may
    # span windows (scatter slices stay within one tap -> unique dsts)
    w_t = {
        t: max(
            1,
            max((len(per_core[c][t][0]) + 127) // 128 for c in range(N_CORES)),
        )
        for t in taps
    }
    col_tap = []
    for t in taps:
        col_tap += [t] * w_t[t]
    W = sum(w_t.values())
    groups = []
    col = 0
    while col < W:
        wg = min(JMAX, W - col)
        runs = []
        a = 0
        while a < wg:
            t = col_tap[col + a]
            b = a
            while b < wg and col_tap[col + b] == t:
                b += 1
            runs.append((t, a, b))
            a = b
        groups.append((wg, runs))
        col += wg
    gidx = np.zeros((N_CORES, W * 128), np.int32)
    sidx = np.full((N_CORES, W * 128), TRASH, np.int32)
    col = 0
    for t in taps:
        for c in range(N_CORES):
            dl, sg = per_core[c][t]
            m = len(dl)
            a = col * 128
            gidx[c, a : a + m] = glob2loc[c, sg]
            sidx[c, a : a + m] = dl
        col += w_t[t]
    assert col == W

    # token i within its 128-col block: column-major packing (token = c*128+p)
    gidx_w = np.stack([wrap16(gidx[c]) for c in range(N_CORES)])
    sidx_w = np.stack([wrap16(sidx[c]) for c in range(N_CORES)])
    return feats_loc, gidx_w, sidx_w, groups, order, SRC_ROWS


def _install_axon_profile_hook():
    try:
        import antenv

        if "antenv.axon_hooks" not in sys.modules:
            mod = types.ModuleType("antenv.axon_hooks")
            hook = [None]
            mod.set_axon_ntff_profile_hook = lambda h: hook.__setitem__(0, h)
            mod.get_axon_ntff_profile_hook = lambda: hook[0]
            sys.modules["antenv.axon_hooks"] = mod
            antenv.axon_hooks = mod
        from antenv.axon_hooks import (
            get_axon_ntff_profile_hook,
            set_axon_ntff_profile_hook,
        )

        if get_axon_ntff_profile_hook() is None:
            from trn_agent_boot.trn_boot import _ntff_profile_via_ctypes

            set_axon_ntff_profile_hook(
                _ntff_profile_via_ctypes("/opt/axon/libaxon_pjrt.so")
            )
    except Exception:
        pass


def build_program(groups, SRC_ROWS, W, do_center=True, ngroups=None, do_scatter=True):
    import os
    n_queues = int(os.environ.get("K_QUEUES", "4"))
    n_tables = int(os.environ.get("K_TABLES", "2"))
    import concourse.bacc as bacc
    import concourse.mybir as mybir
    import concourse.tile as tile
    from concourse.masks import make_identity

    nc = bacc.Bacc(
        "TRN2", num_devices=N_CORES, debug=False, target_bir_lowering=False,
        num_swdge_queues=n_queues,
    )
    f32 = mybir.dt.float32
    i16 = mybir.dt.int16

    bf16 = mybir.dt.bfloat16
    feats_d = nc.dram_tensor("feats_loc", [SRC_ROWS, EP], bf16, kind="ExternalInput").ap()
    ktaps_d = nc.dram_tensor("ktaps", [80, 125 * 80], bf16, kind="ExternalInput").ap()
    gidx_d = nc.dram_tensor("gidx", [128, 8 * W], i16, kind="ExternalInput").ap()
    sidx_d = nc.dram_tensor("sidx", [128, 8 * W], i16, kind="ExternalInput").ap()
    out_d = nc.dram_tensor("out", [NT, EP], f32, kind="ExternalOutput").ap()
    tbl = [
        nc.dram_tensor(f"tbl{i}", [NT, EP], f32, kind="ExternalOutput").ap()
        for i in range(n_tables)
    ]

    with tile.TileContext(nc) as tc:
        with (
            tc.tile_pool(name="const", bufs=1) as cpool,
            tc.tile_pool(name="gath", bufs=3) as gpool,
            tc.tile_pool(name="xts", bufs=4) as xpool,
            tc.tile_pool(name="ysb", bufs=3) as ypool,
            tc.tile_pool(name="xtp", bufs=4, space="PSUM") as xppool,
            tc.tile_pool(name="ypp", bufs=4, space="PSUM") as yppool,
        ):
            ident = cpool.tile([128, 128], bf16)
            make_identity(nc, ident[:])
            ksb = cpool.tile([80, 125 * 80], bf16)
            nc.sync.dma_start(out=ksb[:], in_=ktaps_d[:])
            gsb = cpool.tile([128, 8 * W], i16)
            nc.sync.dma_start(out=gsb[:], in_=gidx_d[:])
            ssb = cpool.tile([128, 8 * W], i16)
            nc.sync.dma_start(out=ssb[:], in_=sidx_d[:])

            def compute_block(G, wg, col_taps, Y):
                """G [128, wg, EP] gathered block -> Y [128, wg, DIM]."""
                for sb in range((wg + SUB - 1) // SUB):
                    k0 = sb * SUB
                    kn = min(SUB, wg - k0)
                    xt_ps = xppool.tile([80, kn, 128], bf16, tag="xtp")
                    for k in range(kn):
                        nc.tensor.transpose(
                            out=xt_ps[:, k, :],
                            in_=G[:, k0 + k, :DIM],
                            identity=ident[:],
                        )
                    xt_sb = xpool.tile([80, kn, 128], bf16, tag="xts")
                    nc.vector.tensor_copy(out=xt_sb[:], in_=xt_ps[:])
                    y_ps = yppool.tile([128, kn, DIM], f32, tag="ypp")
                    for k in range(kn):
                        t = col_taps[k0 + k]
                        nc.tensor.matmul(
                            out=y_ps[:, k, :],
                            lhsT=xt_sb[:, k, :],
                            rhs=ksb[:, t * 80 : (t + 1) * 80],
                            start=True,
                            stop=True,
                        )
                    nc.vector.tensor_copy(out=Y[:, k0 : k0 + kn, :], in_=y_ps[:])

            # ---- center tap: direct DMA both ways --------------------------
            for g in range(NCEN // CJ if do_center else 0):
                r0 = g * CJ * 128
                G = gpool.tile([128, CJ, EP], bf16, tag="G")
                nc.sync.dma_start(
                    out=G[:],
                    in_=feats_d[r0 : r0 + CJ * 128, :].rearrange(
                        "(c p) f -> p c f", p=128
                    ),
                )
                Y = ypool.tile([128, CJ, DIM], f32, tag="Y")
                compute_block(G, CJ, [CENTER_TAP] * CJ, Y)
                nc.sync.dma_start(
                    out=out_d[r0 : r0 + CJ * 128, :DIM].rearrange(
                        "(c p) f -> p c f", p=128
                    ),
                    in_=Y[:],
                )

            # ---- sparse taps: dma_gather + per-tap dma_scatter_add ---------
            n_scat = 0
            col = 0
            for gi_, (wg, gtaps) in enumerate(groups):
                if ngroups is not None and gi_ >= ngroups:
                    break
                i0 = col * 8
                G = gpool.tile([128, wg, EP], bf16, tag="G")
                nc.gpsimd.dma_gather(
                    out_ap=G[:],
                    in_ap=feats_d[:],
                    idxs_ap=gsb[:, i0 : i0 + 8 * wg],
                    num_idxs=wg * 128,
                    num_idxs_reg=wg * 128,
                    elem_size=EP,
                    queue_num=(2 * (gi_ % 2)) % n_queues,
                )
                col_taps = []
                for t, a, b in gtaps:
                    col_taps += [t] * (b - a)
                Y = ypool.tile([128, wg, DIM], f32, tag="Y")
                compute_block(G, wg, col_taps, Y)
                for t, a, b in (gtaps if do_scatter else []):
                    nc.gpsimd.dma_scatter_add(
                        out_ap=tbl[n_scat % n_tables][:, :DIM],
                        in_ap=Y[:, a:b, :],
                        idxs_ap=ssb[:, i0 + 8 * a : i0 + 8 * b],
                        num_idxs=(b - a) * 128,
                        num_idxs_reg=(b - a) * 128,
                        elem_size=DIM,
                        elem_step=EP,
                        queue_num=(1 + 2 * (n_scat % n_tables)) % n_queues if n_queues > 1 else 0,
                    )
                    n_scat += 1
                col += wg
    print("tile build done", file=sys.stderr)
    nc.compile()
    print("bacc compile done", file=sys.stderr)
    return nc


_LAST = {"exec_time_ns": None, "results": None}


def kernel(feats, weight, w_sc0, w_sc1, coords):
    feats = np.ascontiguousarray(np.asarray(feats, np.float32))
    weight = np.asarray(weight, np.float32)
    w_sc0 = np.asarray(w_sc0, np.float32)
    w_sc1 = np.asarray(w_sc1, np.float32)
    coords = np.asarray(coords, np.int32)

    K = make_kernel_np(weight)
    K[CENTER_TAP] = K[CENTER_TAP] + w_sc_embed(w_sc0, w_sc1)
    ktaps = np.ascontiguousarray(K.transpose(1, 0, 2).reshape(80, 125 * 80))

    feats_loc, gidx_w, sidx_w, groups, order, SRC_ROWS = build_plan(feats, coords)
    W = gidx_w.shape[2] // 8
    print(f"plan: W={W} groups={len(groups)} SRC_ROWS={SRC_ROWS}", file=sys.stderr)

    _install_axon_profile_hook()
    from concourse.bass_utils import run_bass_kernel_spmd

    nc = build_program(groups, SRC_ROWS, W)
    import ml_dtypes

    in_maps = [
        {
            "feats_loc": feats_loc[c].astype(ml_dtypes.bfloat16),
            "ktaps": ktaps.astype(ml_dtypes.bfloat16),
            "gidx": gidx_w[c],
            "sidx": sidx_w[c],
        }
        for c in range(N_CORES)
    ]
    import os

    trace = os.environ.get("BASS_KERNEL_TRACE", "0") == "1"
    import time as _time

    res = None
    last_exc = None
    for attempt in range(4):
        try:
            res = run_bass_kernel_spmd(
                nc,
                in_maps,
                core_ids=list(range(N_CORES)),
                trace=trace and attempt == 0,
            )
            break
        except Exception as e:  # device flake: retry, last attempts untraced
            last_exc = e
            print(f"run attempt {attempt} failed: {e}", file=sys.stderr)
            _time.sleep(3.0)
    if res is None:
        raise last_exc
    print("hw run done", file=sys.stderr)
    _LAST["exec_time_ns"] = res.exec_time_ns
    _LAST["results"] = res
    out = np.empty((N, DIM), np.float32)
    for c in range(N_CORES):
        out[order[c * N_LOC : (c + 1) * N_LOC]] = np.asarray(
            res.results[c]["out"]
        )[:N_LOC, :DIM]
    return out

